# revision 29
# baseline (speedup 1.0000x reference)
"""Trainium2 Bass kernel for masked multi-head attention.

Problem (hardcoded): B=2, S=2048, H=16, D_head=64, D_IN=OUT_DIM=1024, fp32 I/O.

Sharding: 8 cores = 2 (batch) x 4 (head-groups of 4 heads). Each core gets its
batch's q/k/v (pre-transposed to [D_IN, S] and cast to bf16 on the host) and
its head-group's weight columns. Each core computes its [S, 256] slice of the
output; the host reassembles the full [B, S, 1024] tensor. No collectives.

Device dataflow, software-pipelined around the ScalarE exp stream:

  4 pair-passes pp = (m-tile mt = pp//2, q-half f = pp%2) covering heads
  hA=2mt (SBUF partitions 0:64) and hB=2mt+1 (partitions 64:128).  The
  K=64 score matmuls of the two heads map onto the PE's 64-row tiles
  T0/T8 (via lhsT/rhs base partitions), so interleaving A/B score
  matmuls streams them CONCURRENTLY (~2.4x faster than one head alone,
  measured on HW).

  per kt: scoresT_A/B [128,1024] psum, expT_A/B = exp(. + mask bias),
          A@V for pass pp-1: head hA drained at 2 k-tiles/iter during
          iters 0-7 into psum accumulator pqA, head hB during iters
          8-15 into pqB; whichever accumulator is idle serves
          background projections (v-proj, m-tile-1 k/q blocks).
  [U^T; D] += [vw_h | ones]^T @ expT(kt): full-util [128,128]
          stationary; U rows 0:64, D (denominator) replicated 64:128.
  out = U^T * qmask / D on VectorE, DMA'd per 512-block.

  Input DMA is piece-wise (k/v in 512-col blocks interleaved, q in
  1024-col halves) with projections trailing arrival; expT tiles live
  in two rings (18 A-slots + 24 B-slots of [128,1024] bf16) consumed in
  production order by the lagged A@V streams.
"""

import sys
import numpy as np

sys.path.insert(0, "/opt/trn_rl_repo")

import ml_dtypes

BF16 = np.dtype(ml_dtypes.bfloat16)

B = 2
S = 2048
H = 16
DH = 64
D_IN = 1024
OUT_DIM = 1024
N_CORES = 8
HEADS_PER_CORE = 4
MCOLS = HEADS_PER_CORE * DH  # 256
MASK_NEG = -30000.0


def build_nc(s=S, reps=1):
    """Build the single-core Bass graph (SPMD: same graph on all 8 cores)."""
    import concourse.bass as bass
    import concourse.bacc as bacc
    import concourse.tile as tile
    from concourse import mybir
    from contextlib import ExitStack

    f32 = mybir.dt.float32
    bf16 = mybir.dt.bfloat16

    nkt = s // 128          # 16 scoresT partition tiles along k
    nch = D_IN // 128       # 8  d_in chunks
    hw = s // 2             # 1024 q-half width
    NPP = 4                 # pair passes
    ERA, ERB = 17, 24       # expT ring slots per head-parity (exact lower bounds; SBUF is full)

    nc = bacc.Bacc("TRN2", target_bir_lowering=False, debug=False,
                   num_devices=N_CORES)

    qT_ext = nc.dram_tensor("qT", [D_IN, s], bf16, kind="ExternalInput").ap()
    kT_ext = nc.dram_tensor("kT", [D_IN, s], bf16, kind="ExternalInput").ap()
    vT_ext = nc.dram_tensor("vT", [D_IN, s], bf16, kind="ExternalInput").ap()
    # weights are pre-rearranged on the host to [128, nch*MCOLS] so the
    # load is one dense 4KB-per-partition transfer instead of a strided
    # gather of 512B segments (which measured ~2x slower on HW)
    wq_ext = nc.dram_tensor("wq", [128, D_IN // 128 * MCOLS], bf16,
                            kind="ExternalInput").ap()
    wk_ext = nc.dram_tensor("wk", [128, D_IN // 128 * MCOLS], bf16,
                            kind="ExternalInput").ap()
    wv_ext = nc.dram_tensor("wv", [128, D_IN // 128 * MCOLS], bf16,
                            kind="ExternalInput").ap()
    mb_ext = nc.dram_tensor("mb", [128, nkt], f32, kind="ExternalInput").ap()
    qm_ext = nc.dram_tensor("qm", [1, s], f32, kind="ExternalInput").ap()
    out_ext = nc.dram_tensor("out", [MCOLS, s], f32, kind="ExternalOutput").ap()

    Exp = mybir.ActivationFunctionType.Exp

    with tile.TileContext(nc) as tc:
        with ExitStack() as ctx:
            misc = ctx.enter_context(tc.tile_pool(name="misc", bufs=1))
            wpool = ctx.enter_context(tc.tile_pool(name="wpool", bufs=1))
            xqp = ctx.enter_context(tc.tile_pool(name="xqp", bufs=1))
            xkp = ctx.enter_context(tc.tile_pool(name="xkp", bufs=1))
            xvp = ctx.enter_context(tc.tile_pool(name="xvp", bufs=1))
            qkw = ctx.enter_context(tc.tile_pool(name="qkw", bufs=1))
            vwp = ctx.enter_context(tc.tile_pool(name="vwp", bufs=1))
            expp = ctx.enter_context(tc.tile_pool(name="expp", bufs=1))
            scp = ctx.enter_context(tc.tile_pool(name="scp", bufs=1))
            outp = ctx.enter_context(tc.tile_pool(name="outp", bufs=2))
            psS = ctx.enter_context(tc.tile_pool(name="psS", bufs=1, space="PSUM"))
            psQ = ctx.enter_context(tc.tile_pool(name="psQ", bufs=1, space="PSUM"))

            for _rep in range(reps):
                # ================= DMA stream (SP FIFO order) =================
                mb_sb = misc.tile([128, nkt], f32, tag="mb")
                nc.sync.dma_start(out=mb_sb[:], in_=mb_ext[:])

                w_sb = {}
                for wnm, ext in (("wq", wq_ext), ("wk", wk_ext), ("wv", wv_ext)):
                    wt = wpool.tile([128, nch, MCOLS], bf16, name=wnm, tag=wnm)
                    nc.sync.dma_start(
                        out=wt[:], in_=ext.rearrange("p (c m) -> p c m", c=nch))
                    w_sb[wnm] = wt

                xq_t = {}
                xk_t = {}
                xk23_t = {}
                xv_t = {}

                def dma_q(h):
                    for c in range(nch):
                        t = xqp.tile([128, hw], bf16, tag=f"xq{c}", name="xq")
                        nc.sync.dma_start(
                            out=t[:],
                            in_=qT_ext[c * 128:(c + 1) * 128, h * hw:(h + 1) * hw])
                        xq_t[(h, c)] = t

                def dma_k512(b):
                    """k columns [512b, 512b+512) as 8 x [128,512] pieces."""
                    for c in range(nch):
                        t = xkp.tile([128, 512], bf16, tag=f"xk{c}",
                                     name="xk")
                        nc.sync.dma_start(
                            out=t[:],
                            in_=kT_ext[c * 128:(c + 1) * 128,
                                       b * 512:(b + 1) * 512])
                        xk_t[(b, c)] = t

                def dma_wide(which, ext, pool, store, b2):
                    """columns [1024*b2, +1024) as 8 x [128,1024] pieces
                    (4KB-per-partition class transfers are ~1.6x more
                    bandwidth-efficient than 1KB ones on HW)."""
                    for c in range(nch):
                        t = pool.tile([128, 1024], bf16, tag=f"{which}{c}",
                                      name=which)
                        nc.sync.dma_start(
                            out=t[:],
                            in_=ext[c * 128:(c + 1) * 128,
                                    b2 * 1024:(b2 + 1) * 1024])
                        store[(b2, c)] = t

                # order: qh0, k0, k1, v-lo, k-hi, v-hi, qh1 — k blocks must
                # land just ahead of the pass-0 score stream (kt4 needs k1,
                # kt8 needs k-hi); v trails into the background v-proj slots
                dma_q(0)
                dma_k512(0)
                dma_k512(1)
                dma_wide("xv", vT_ext, xvp, xv_t, 0)
                dma_wide("xk2", kT_ext, xkp, xk23_t, 1)
                dma_wide("xv", vT_ext, xvp, xv_t, 1)
                dma_q(1)

                qm_bc = misc.tile([64, s], f32, tag="qm")
                qm_ap = qm_ext[:]
                qm_src = bass.AP(tensor=qm_ap.tensor, offset=qm_ap.offset,
                                 ap=[[0, 64]] + qm_ap.ap[1:])
                nc.sync.dma_start(out=qm_bc[:], in_=qm_src)

                # ================= constants / warmup =================
                warm = misc.tile([1, 2], f32, tag="warm")
                nc.vector.memset(warm[:], 0.0)
                nc.scalar.activation(warm[:], warm[:], Exp, bias=0.0, scale=1.0)

                vw = vwp.tile([128, nkt, HEADS_PER_CORE, 128], bf16, tag="vw")
                nc.vector.memset(vw[:, :, :, DH:128], 1.0)

                qwT = qkw.tile([128, 2, s], bf16, tag="qwT")
                kwT = qkw.tile([128, 2, s], bf16, tag="kwT")

                # ================= projection building blocks =================
                def q_proj(h, mt, pool, tag):
                    """One [128,1024] group (two bank-aligned half-columns):
                    half h x m-tile mt."""
                    g = pool.tile([128, 1024], f32, tag=tag, name="gq")
                    for c in range(nch):
                        for j in range(2):
                            nc.tensor.matmul(
                                g[:, j * 512:(j + 1) * 512],
                                w_sb["wq"][:, c, mt * 128:(mt + 1) * 128],
                                xq_t[(h, c)][:, j * 512:(j + 1) * 512],
                                start=(c == 0), stop=(c == nch - 1))
                    nc.vector.tensor_copy(
                        qwT[:, mt, h * hw:(h + 1) * hw], g[:, :])

                def k_proj(b, mt, tag):
                    g = psQ.tile([128, 512], f32, tag=tag, name="gk")
                    for c in range(nch):
                        rhs = (xk_t[(b, c)][:, :] if b < 2 else
                               xk23_t[(1, c)][:, (b - 2) * 512:(b - 1) * 512])
                        nc.tensor.matmul(
                            g[:, :],
                            w_sb["wk"][:, c, mt * 128:(mt + 1) * 128],
                            rhs,
                            start=(c == 0), stop=(c == nch - 1))
                    nc.vector.tensor_copy(
                        kwT[:, mt, b * 512:(b + 1) * 512], g[:, :])

                def v_proj_st(st, tag):
                    b2 = st // 8
                    si = st - b2 * 8
                    g = psQ.tile([128, 256], f32, tag=tag, name="gv")
                    for c in range(nch):
                        nc.tensor.matmul(
                            g[:, :],
                            xv_t[(b2, c)][:, si * 128:(si + 1) * 128],
                            w_sb["wv"][:, c, :],
                            start=(c == 0), stop=(c == nch - 1))
                    for hh in range(HEADS_PER_CORE):
                        nc.vector.tensor_copy(
                            vw[:, st, hh, 0:DH],
                            g[:, hh * DH:(hh + 1) * DH])

                # ================= attention machinery =================
                er_idx = [0, 0]
                exp_ring = {}    # (pp, hpar, kt) -> tile

                def new_exp_tile(pp, hpar, kt):
                    ring = (ERA, ERB)[hpar]
                    t = expp.tile([128, 1024], bf16,
                                  tag=f"er{hpar}_{er_idx[hpar] % ring}",
                                  name="et")
                    er_idx[hpar] += 1
                    exp_ring[(pp, hpar, kt)] = t
                    return t

                pq_tiles = {}    # (pp, hpar) -> tile [128, 1024]

                def av_step(pp, hpar, kt):
                    h = 2 * (pp // 2) + hpar
                    if kt == 0:
                        pq_tiles[(pp, hpar)] = psQ.tile(
                            [128, 1024], f32,
                            tag=("pqA", "pqB")[hpar], name="pQ")
                    et = exp_ring.pop((pp, hpar, kt))
                    pq = pq_tiles[(pp, hpar)]
                    for j in range(2):
                        nc.tensor.matmul(
                            pq[:, j * 512:(j + 1) * 512],
                            vw[:, kt, h, :],
                            et[:, j * 512:(j + 1) * 512],
                            start=(kt == 0), stop=(kt == nkt - 1))

                def norm(pp, hpar):
                    mt, f = pp // 2, pp % 2
                    h = 2 * mt + hpar
                    pq = pq_tiles.pop((pp, hpar))
                    den = scp.tile([64, hw], f32, tag="den")
                    sc = scp.tile([64, hw], f32, tag="sc")
                    nc.vector.tensor_copy(den[:, :], pq[64:128, :])
                    nc.vector.reciprocal_approx_fast(sc[:, :], den[:, :])
                    nc.vector.tensor_mul(sc[:, :], sc[:, :],
                                         qm_bc[:, f * hw:(f + 1) * hw])
                    ot = outp.tile([64, hw], f32, tag="osb", name="ot")
                    nc.vector.tensor_mul(ot[:], pq[0:64, :], sc[:, :])
                    nc.sync.dma_start(
                        out=out_ext[h * DH:(h + 1) * DH, f * hw:(f + 1) * hw],
                        in_=ot[:])

                # ---- background schedule ----
                # Each entry runs at the top of (pass, kt).  The idle A@V
                # accumulator tag (pqA during iters 8-15, pqB during 0-7)
                # hosts background groups; within pass 0 both are free.
                qh1_state = {}

                def qh1_part(mt, tag, lo, hi):
                    """q-half1 proj for m-tile mt, chunks [lo, hi)."""
                    if mt not in qh1_state:
                        qh1_state[mt] = psQ.tile([128, 1024], f32, tag=tag,
                                                 name="gq1")
                    g = qh1_state[mt]
                    for c in range(lo, hi):
                        for j in range(2):
                            nc.tensor.matmul(
                                g[:, j * 512:(j + 1) * 512],
                                w_sb["wq"][:, c, mt * 128:(mt + 1) * 128],
                                xq_t[(1, c)][:, j * 512:(j + 1) * 512],
                                start=(c == 0), stop=(c == nch - 1))
                    if hi == nch:
                        nc.vector.tensor_copy(qwT[:, mt, hw:2 * hw], g[:, :])
                        del qh1_state[mt]

                bg = {}
                bg[(0, 2)] = [lambda: k_proj(1, 0, "pqA")]
                bg[(0, 3)] = [lambda: k_proj(1, 1, "pqB")]
                bg[(0, 4)] = [lambda: v_proj_st(0, "pqA"),
                              lambda: v_proj_st(1, "pqB")]
                bg[(0, 5)] = [lambda: v_proj_st(2, "pqA"),
                              lambda: v_proj_st(3, "pqB")]
                bg[(0, 6)] = [lambda: v_proj_st(4, "pqA"),
                              lambda: v_proj_st(5, "pqB")]
                bg[(0, 7)] = [lambda: v_proj_st(6, "pqA"),
                              lambda: v_proj_st(7, "pqB")]
                bg[(0, 8)] = [lambda: k_proj(2, 0, "pqA")]
                bg[(0, 9)] = [lambda: k_proj(3, 0, "pqB")]
                bg[(0, 11)] = [lambda: v_proj_st(8, "pqA"),
                               lambda: v_proj_st(9, "pqB")]
                bg[(0, 12)] = [lambda: v_proj_st(10, "pqA"),
                               lambda: v_proj_st(11, "pqB")]
                bg[(0, 13)] = [lambda: v_proj_st(12, "pqA"),
                               lambda: v_proj_st(13, "pqB")]
                bg[(0, 14)] = [lambda: qh1_part(0, "pqA", 0, 4)]
                bg[(0, 15)] = [lambda: qh1_part(0, "pqA", 4, 8)]
                # pass 1: pqB free iters 0-7 (st14/15 must land by iter 7:
                # the A-drain reaches kt15 then), pqA free after iter-8 norm
                bg[(1, 0)] = [lambda: v_proj_st(14, "pqB")]
                bg[(1, 1)] = [lambda: v_proj_st(15, "pqB")]
                bg[(1, 11)] = [lambda: k_proj(2, 1, "pqA")]
                # pass 2: pqB free iters 0-7
                bg[(2, 0)] = [lambda: k_proj(3, 1, "pqB")]
                bg[(2, 2)] = [lambda: qh1_part(1, "pqB", 0, 4)]
                bg[(2, 3)] = [lambda: qh1_part(1, "pqB", 4, 8)]

                # ================= emit =================
                q_proj(0, 0, psS, "psA")
                q_proj(0, 1, psS, "psB")
                k_proj(0, 0, "pqA")
                k_proj(0, 1, "pqB")

                for pp in range(NPP):
                    mt, f = pp // 2, pp % 2
                    for kt in range(nkt):
                        if pp >= 1 and kt == 8:
                            norm(pp - 1, 0)
                        for fn in bg.get((pp, kt), ()):
                            fn()
                        psA = psS.tile([128, 1024], f32, tag="psA", name="psA")
                        psB = psS.tile([128, 1024], f32, tag="psB", name="psB")
                        # head A's two matmuls first so exp-A's input is
                        # complete one matmul earlier; T8 still overlaps T0
                        # via the 64-deep PE queue
                        for hpar, pst in ((0, psA), (1, psB)):
                            hp = 64 * hpar
                            for j in range(2):
                                qb = 2 * f + j
                                nc.tensor.matmul(
                                    pst[:, j * 512:(j + 1) * 512],
                                    kwT[hp:hp + 64, mt, kt * 128:(kt + 1) * 128],
                                    qwT[hp:hp + 64, mt, qb * 512:(qb + 1) * 512],
                                    start=True, stop=True)
                        if pp >= 1:
                            if kt < 8:
                                av_step(pp - 1, 0, 2 * kt)
                                av_step(pp - 1, 0, 2 * kt + 1)
                            else:
                                av_step(pp - 1, 1, 2 * (kt - 8))
                                av_step(pp - 1, 1, 2 * (kt - 8) + 1)
                        etA = new_exp_tile(pp, 0, kt)
                        nc.scalar.activation(
                            etA[:], psA[:], Exp,
                            bias=mb_sb[:, kt:kt + 1], scale=1.0)
                        etB = new_exp_tile(pp, 1, kt)
                        nc.scalar.activation(
                            etB[:], psB[:], Exp,
                            bias=mb_sb[:, kt:kt + 1], scale=1.0)
                    if pp >= 1:
                        norm(pp - 1, 1)

                # tail: A@V + norm for the last pair pass
                for hpar in (0, 1):
                    for kt in range(nkt):
                        av_step(NPP - 1, hpar, kt)
                    norm(NPP - 1, hpar)

    nc.compile()
    return nc


def _wprep(w):
    """[1024, 256] -> [128, 8*256]: chunk-major layout for a dense DMA."""
    nch = D_IN // 128
    return np.ascontiguousarray(
        np.asarray(w).reshape(nch, 128, MCOLS).transpose(1, 0, 2)
        .reshape(128, nch * MCOLS)).astype(BF16)


def shard_inputs(q, k, v, v_mask, q_mask, Wq, Wk, Wv, s=S):
    """Host-side sharding: core i -> (batch i//4, head-group i%4)."""
    scale = np.float32(1.0 / np.sqrt(DH))
    nkt = s // 128
    in_maps = []
    qT = [np.ascontiguousarray(np.asarray(q)[b, :s].T).astype(BF16) for b in range(B)]
    kT = [np.ascontiguousarray(np.asarray(k)[b, :s].T).astype(BF16) for b in range(B)]
    vT = [np.ascontiguousarray(np.asarray(v)[b, :s].T).astype(BF16) for b in range(B)]
    mb = []
    qm = []
    for b in range(B):
        bias = np.where(np.asarray(v_mask)[b, :s, 0] > 0.5, 0.0,
                        MASK_NEG).astype(np.float32)
        mb.append(np.ascontiguousarray(bias.reshape(nkt, 128).T))  # [128, nkt]
        qm.append(np.ascontiguousarray(
            np.asarray(q_mask)[b, :s, 0].reshape(1, s).astype(np.float32)))
    Wq = np.asarray(Wq)
    Wk = np.asarray(Wk)
    Wv = np.asarray(Wv)
    for i in range(N_CORES):
        b, g = divmod(i, HEADS_PER_CORE)
        cols = slice(g * MCOLS, (g + 1) * MCOLS)
        in_maps.append({
            "qT": qT[b],
            "kT": kT[b],
            "vT": vT[b],
            "wq": _wprep(Wq[:, cols] * scale),
            "wk": _wprep(Wk[:, cols]),
            "wv": _wprep(Wv[:, cols]),
            "mb": mb[b],
            "qm": qm[b],
        })
    return in_maps


_CACHED = {}


def _get_compiled(s=S):
    if s not in _CACHED:
        _CACHED[s] = build_nc(s)
    return _CACHED[s]


def kernel(q, k, v, v_mask, q_mask, Wq, Wk, Wv):
    from concourse.bass_utils import run_bass_kernel_spmd

    nc = _get_compiled(S)
    in_maps = shard_inputs(q, k, v, v_mask, q_mask, Wq, Wk, Wv, S)
    res = run_bass_kernel_spmd(nc, in_maps, core_ids=list(range(N_CORES)))
    out = np.empty((B, S, OUT_DIM), dtype=np.float32)
    for i in range(N_CORES):
        b, g = divmod(i, HEADS_PER_CORE)
        out[b, :, g * MCOLS:(g + 1) * MCOLS] = res.results[i]["out"].T
    return out
